# revision 39
# baseline (speedup 1.0000x reference)
"""CrossViewAttention Trainium2 kernel (fp8-DoubleRow scores + bf16 output).

Math: for each batch row b with features f1, f2 (D=1024):
  Q_s = f_s Wq^T + bq ; K_t = f_t Wk^T + bk ; V_t = f_t Wv^T + bv
  scores s_st = Q_s.K_t / sqrt(D); attn = softmax over t; out = sum_s attn_st V_t

2-way softmax collapses to sigmoids of score differences:
  d1 = (s11-s12) = (f1.(g @ M^T) + g.ck)/sqrt(D)
  d2 = (s21-s22) = (f2.(g @ M^T) + g.ck)/sqrt(D)
  with g = f1-f2, M = Wq^T Wk, ck = Wk^T bq  (bk and bq-cross terms cancel)
  w1 = sigmoid(d1)+sigmoid(d2); w2 = 2-w1
  out = (w1*f1 + w2*f2) @ Wv^T + 2*bv

Per 128-row chunk only TWO 128x1024x1024 matmuls are needed:
  mm1 (scores; error-tolerant so fp8e4m3 DoubleRow: 256-deep contraction at
       0.5 cycles/row, 4x the fp32r rate)
  mm2 (output): bf16, plus 8 PE block-transposes to form X^T.
f1/f2/X/out are bf16; the +2*bv bias and final f32 upcast happen on host,
as do the weight transforms (M^T, ck) and the g^T fp8 tiles.

Steady state is PE-bound at 4693 cycles-equivalent/chunk with a four-stage
software pipeline (mm1(i) | transpose(i-2) | mm2(i-3) | store(i-4)) so the
PE never waits on the DVE/Act sigmoid chain, and each in-order engine
queue (Act, SP) receives its ops in ready-time order:
  Act: ud->bf16 copy(i), xt copy(i-2), sigmoid(i), ob copies(i-3)
  SP:  f1/f2/g^T loads(i), out store(i-4)
The large Wv load is deferred off the startup critical path, and a junk
matmul burst pre-ramps the tensor engine p-state during the prologue.

Sharding: batch split across 8 cores (2048 rows each), weights replicated.
"""

import sys

for _p in ("/opt/trn_rl_repo",):
    if _p not in sys.path:
        sys.path.insert(0, _p)

import numpy as np
import ml_dtypes

import concourse.bacc as bacc
import concourse.mybir as mybir
import concourse.tile as tile

F32 = mybir.dt.float32
BF16 = mybir.dt.bfloat16
FP8 = mybir.dt.float8e4

B = 16384
D = 1024
NCORES = 8
R = B // NCORES          # rows per core
CH = 128                 # chunk rows
KT = D // 128            # contraction k-tiles (8)
SCALE = 1.0 / float(np.sqrt(D))
GS = 4.0                 # fp8 pre-scale on g
MS = 16.0                # fp8 pre-scale on M^T
SCALE2 = SCALE / (GS * MS)
DR = mybir.MatmulPerfMode.DoubleRow


def build(nc, n_chunks, repeats=1):
    f1s = nc.dram_tensor("f1s", [n_chunks * CH, D], BF16, kind="ExternalInput").ap()
    f2s = nc.dram_tensor("f2s", [n_chunks * CH, D], BF16, kind="ExternalInput").ap()
    gtb = nc.dram_tensor("gtb", [n_chunks, 128, KT, CH], FP8, kind="ExternalInput").ap()
    gckb = nc.dram_tensor("gckb", [128, n_chunks], F32, kind="ExternalInput").ap()
    mtb = nc.dram_tensor("mtb", [128, KT, D], FP8, kind="ExternalInput").ap()
    wvt = nc.dram_tensor("wvt", [128, KT, D], BF16, kind="ExternalInput").ap()
    idn = nc.dram_tensor("idn", [128, 128], BF16, kind="ExternalInput").ap()
    out = nc.dram_tensor("out", [n_chunks * CH, D], BF16, kind="ExternalOutput").ap()

    with tile.TileContext(nc) as tc:
        with (
            tc.tile_pool(name="wpool", bufs=1) as wpool,
            tc.tile_pool(name="io", bufs=3) as io,
            tc.tile_pool(name="work", bufs=2) as work,
            tc.tile_pool(name="xrpool", bufs=3) as xrpool,
            tc.tile_pool(name="small", bufs=2) as small,
            tc.tile_pool(name="ps_ud", bufs=1, space="PSUM") as ps_ud,
            tc.tile_pool(name="ps_xt", bufs=1, space="PSUM") as ps_xt,
            tc.tile_pool(name="ps_o", bufs=2, space="PSUM") as ps_o,
        ):
            # resident weights. mt_sb is needed by mm1(0) immediately; wv_sb
            # only by mm2 three periods in, so its (large) DMA is deferred
            # into the loop to keep it off the startup critical path.
            mt_sb = wpool.tile([128, KT, D], FP8)
            nc.sync.dma_start(mt_sb[:, :, :], mtb[:, :, :])
            gck_sb = wpool.tile([128, n_chunks], F32)
            nc.sync.dma_start(gck_sb[:], gckb[:])
            id_sb = wpool.tile([128, 128], BF16)
            nc.sync.dma_start(id_sb[:], idn[:])
            wv_sb = wpool.tile([128, KT, D], BF16)
            wv_loaded = [False]

            def load_wv():
                if not wv_loaded[0]:
                    nc.sync.dma_start(wv_sb[:, :, :], wvt[:, :, :])
                    wv_loaded[0] = True

            # warm up the PE during the prologue DMAs: the tensor engine
            # p-state ramps to full clock only after ~3us of continuous busy,
            # so a burst of junk matmuls before mm1(0) keeps the real work
            # from running at the mid/low p-state.
            junk = wpool.tile([128, 512], BF16)
            nc.vector.memset(junk[:], 0.0)
            jps = ps_xt.tile([128, 512], F32, tag="warm")
            for w in range(4):
                nc.tensor.matmul(
                    jps[:], junk[:, 0:128], junk[:], start=w == 0, stop=w == 3
                )

            idxs = [it % n_chunks for it in range(n_chunks * repeats)]
            # four-stage software pipeline:
            #   transpose (XBAR) at skew-2, mm2 at skew-3, store at skew-4
            pend_t = []  # [(chunk_index, xr_tile), ...] awaiting transpose
            pend_m = []  # [(chunk_index, xt_tile), ...] awaiting mm2
            pend_o = []  # [(chunk_index, ob_tile), ...] awaiting store

            def stage_t(i, xr):
                """PE block-transposes of X, then Act copy psum->sbuf."""
                xt_ps = ps_xt.tile([128, D], BF16, tag="xt")
                for k in range(KT):
                    nc.tensor.transpose(
                        xt_ps[:, k * 128 : (k + 1) * 128],
                        xr[:, k * 128 : (k + 1) * 128],
                        id_sb[:],
                    )
                xt = work.tile([128, D], BF16, tag="xts")
                nc.scalar.copy(xt[:], xt_ps[:])
                return xt

            def stage_m_mm(i, xt):
                """mm2 from the already-transposed xt (PE only)."""
                # two single-bank psum halves so each frees right after its copy
                po_a = ps_o.tile([128, 512], F32, tag="poa")
                po_b = ps_o.tile([128, 512], F32, tag="pob")
                for k in range(KT):
                    lhs = xt[:, k * 128 : (k + 1) * 128]
                    st = k == 0
                    sp = k == KT - 1
                    nc.tensor.matmul(
                        po_a[:], lhs, wv_sb[:, k, 0:512], start=st, stop=sp
                    )
                    nc.tensor.matmul(
                        po_b[:], lhs, wv_sb[:, k, 512:1024], start=st, stop=sp
                    )
                return po_a, po_b

            def stage_m_ob(i, po_a, po_b, split=False):
                """psum -> bf16 sbuf (Act), emitted after sigmoid(i) so the
                in-order Act queue stays in ready-time order. With split=True
                (pipeline drain) the DVE takes one half to shorten the tail."""
                ob = work.tile([128, D], BF16, tag="ob")
                nc.scalar.copy(ob[:, 0:512], po_a[:])
                if split:
                    nc.vector.tensor_scalar(
                        ob[:, 512:1024], po_b[:], 1.0, None,
                        op0=mybir.AluOpType.mult,
                    )
                else:
                    nc.scalar.copy(ob[:, 512:1024], po_b[:])
                return ob

            def stage_o(i, ob):
                rs = i * CH
                nc.sync.dma_start(out[rs : rs + CH, :], ob[:])

            first = [True]
            for i in idxs:
                rs = i * CH
                # ---- loads (first iteration goes via the idle Act queue so
                #      it runs concurrently with the mt_sb issue on SP)
                ld = nc.scalar if first[0] else nc.sync
                first[0] = False
                f1t = io.tile([128, D], BF16, tag="f1t")
                ld.dma_start(f1t[:], f1s[rs : rs + CH, :])
                f2t = io.tile([128, D], BF16, tag="f2t")
                ld.dma_start(f2t[:], f2s[rs : rs + CH, :])
                gt = io.tile([128, KT, CH], FP8, tag="gt")
                ld.dma_start(gt[:, :, :], gtb[i, :, :, :])
                load_wv()

                # ---- mm1: Ud = g @ M^T  (fp8 DoubleRow, 256-deep per instr)
                ud = ps_ud.tile([128, D], F32, tag="ud")
                for kp in range(KT // 2):
                    st = kp == 0
                    sp = kp == KT // 2 - 1
                    lhs = gt[:, 2 * kp : 2 * kp + 2, :]
                    nc.tensor.matmul(
                        ud[:, 0:512],
                        lhs,
                        mt_sb[:, 2 * kp : 2 * kp + 2, 0:512],
                        start=st,
                        stop=sp,
                        perf_mode=DR,
                    )
                    nc.tensor.matmul(
                        ud[:, 512:1024],
                        lhs,
                        mt_sb[:, 2 * kp : 2 * kp + 2, 512:1024],
                        start=st,
                        stop=sp,
                        perf_mode=DR,
                    )

                # ---- Ud -> bf16 SBUF first in the Act queue (earliest ready),
                #      then dots d_s = sum(f_s*Ud)*SCALE2
                #      (g.ck/sqrt(D) enters as the sigmoid's bias below)
                ud_sb = work.tile([128, D], BF16, tag="udsb")
                nc.scalar.copy(ud_sb[:], ud[:])
                dd = small.tile([128, 2], F32, tag="dd")
                scr1 = work.tile([128, D], BF16, tag="scr")
                nc.vector.scalar_tensor_tensor(
                    out=scr1[:],
                    in0=f1t[:],
                    scalar=SCALE2,
                    in1=ud_sb[:],
                    op0=mybir.AluOpType.mult,
                    op1=mybir.AluOpType.mult,
                    accum_out=dd[:, 0:1],
                )
                scr2 = work.tile([128, D], BF16, tag="scr")
                nc.vector.scalar_tensor_tensor(
                    out=scr2[:],
                    in0=f2t[:],
                    scalar=SCALE2,
                    in1=ud_sb[:],
                    op0=mybir.AluOpType.mult,
                    op1=mybir.AluOpType.mult,
                    accum_out=dd[:, 1:2],
                )

                # ---- overlap: PE transposes chunk i-2 (+ Act xt copy, which
                #      lands after the ud copy in the Act queue), mm2 chunk
                #      i-3 on the PE behind the transposes.
                if len(pend_t) >= 2:
                    ti, txr = pend_t.pop(0)
                    pend_m.append((ti, stage_t(ti, txr)))
                mm_done = None
                if len(pend_m) >= 2:
                    mi, mxt = pend_m.pop(0)
                    mm_done = (mi, *stage_m_mm(mi, mxt))

                # ---- w1 = sig(d1+gck)+sig(d2+gck) in ONE Act op (accum_out)
                sg = small.tile([128, 2], F32, tag="sg")
                w1 = small.tile([128, 1], F32, tag="w1")
                nc.scalar.activation(
                    sg[:],
                    dd[:],
                    mybir.ActivationFunctionType.Sigmoid,
                    bias=gck_sb[:, i : i + 1],
                    accum_out=w1[:],
                )
                w2 = small.tile([128, 1], F32, tag="w2")
                nc.vector.tensor_scalar(
                    w2[:],
                    w1[:],
                    -1.0,
                    2.0,
                    op0=mybir.AluOpType.mult,
                    op1=mybir.AluOpType.add,
                )

                # ---- X = w1*f1 + w2*f2   (bf16, all 4x-mode TSP + 2x TT)
                t1 = work.tile([128, D], BF16, tag="t1")
                nc.vector.tensor_scalar(
                    t1[:], f2t[:], w2[:], None, op0=mybir.AluOpType.mult
                )
                t2 = work.tile([128, D], BF16, tag="t2")
                nc.vector.tensor_scalar(
                    t2[:], f1t[:], w1[:], None, op0=mybir.AluOpType.mult
                )
                xr = xrpool.tile([128, D], BF16, tag="xr")
                nc.vector.tensor_tensor(
                    xr[:], t1[:], t2[:], op=mybir.AluOpType.add
                )
                pend_t.append((i, xr))

                # ---- ob copies for the mm2 issued above, after sigmoid(i)
                #      in the Act queue (ready-time order), then the store
                #      issue for chunk i-4 (its ob has long been ready)
                if mm_done is not None:
                    oi, pa, pb = mm_done
                    pend_o.append((oi, stage_m_ob(oi, pa, pb)))
                if len(pend_o) >= 2:
                    stage_o(*pend_o.pop(0))

            # ---- drain the pipeline
            for ti, txr in pend_t:
                pend_m.append((ti, stage_t(ti, txr)))
            for mi, mxt in pend_m:
                pa, pb = stage_m_mm(mi, mxt)
                pend_o.append((mi, stage_m_ob(mi, pa, pb, split=True)))
            for p in pend_o:
                stage_o(*p)

    return out


_CACHE = {}


def get_compiled(n_chunks=R // CH):
    key = n_chunks
    if key not in _CACHE:
        nc = bacc.Bacc(
            "TRN2", target_bir_lowering=False, debug=False, num_devices=NCORES
        )
        build(nc, n_chunks)
        nc.compile()
        _CACHE[key] = nc
    return _CACHE[key]


def prep_inputs(f1, f2, Wq, bq, Wk, bk, Wv, bv):
    """Host-side algebra + sharding. Returns per-core input maps."""
    f1 = np.ascontiguousarray(np.asarray(f1), dtype=np.float32)
    f2 = np.ascontiguousarray(np.asarray(f2), dtype=np.float32)
    Wq = np.asarray(Wq, dtype=np.float32)
    bq = np.asarray(bq, dtype=np.float32)
    Wk = np.asarray(Wk, dtype=np.float32)
    Wv = np.asarray(Wv, dtype=np.float32)
    g = f1 - f2

    WkT = np.ascontiguousarray(Wk.T)
    MT = WkT @ Wq                             # M^T = Wk^T Wq  [D, D]
    ck = WkT @ bq                             # [D]
    gck = (g @ ck) * np.float32(SCALE)        # [B]
    mtb = np.ascontiguousarray(
        (MT * MS).astype(ml_dtypes.float8_e4m3).reshape(KT, 128, D).transpose(1, 0, 2)
    )
    wvt = np.ascontiguousarray(
        Wv.T.astype(ml_dtypes.bfloat16).reshape(KT, 128, D).transpose(1, 0, 2)
    )
    idn = np.eye(128, dtype=ml_dtypes.bfloat16)
    g8 = (g * GS).astype(ml_dtypes.float8_e4m3)
    f1b = f1.astype(ml_dtypes.bfloat16)
    f2b = f2.astype(ml_dtypes.bfloat16)

    n_chunks = R // CH
    in_maps = []
    for c in range(NCORES):
        sl = slice(c * R, (c + 1) * R)
        gtb = np.ascontiguousarray(
            g8[sl].reshape(n_chunks, CH, KT, 128).transpose(0, 3, 2, 1)
        )
        gckb = np.ascontiguousarray(gck[sl].reshape(n_chunks, CH).T)
        in_maps.append(
            {
                "f1s": np.ascontiguousarray(f1b[sl]),
                "f2s": np.ascontiguousarray(f2b[sl]),
                "gtb": gtb,
                "gckb": gckb,
                "mtb": mtb,
                "wvt": wvt,
                "idn": idn,
            }
        )
    return in_maps


def kernel(**inputs):
    from concourse.bass_utils import run_bass_kernel_spmd

    nc = get_compiled()
    in_maps = prep_inputs(**inputs)
    res = run_bass_kernel_spmd(nc, in_maps, core_ids=list(range(NCORES)))
    bv = np.asarray(inputs["bv"], dtype=np.float32)
    out = np.concatenate(
        [res.results[c]["out"] for c in range(NCORES)], axis=0
    ).astype(np.float32)
    return out + 2.0 * bv
